# revision 22
# baseline (speedup 1.0000x reference)
"""Trainium2 Bass kernel for CrossAttention (LN -> QKV proj -> MHA -> out proj).

Sharding: data-parallel over (batch, query-half): 8 shards for B=4.
Each core gets a [1024, 1024] query-token slice and the full [2048, 768]
context for its batch, and produces a [1024, 1024] output slice.

v2 design notes (engine-stream oriented; the ACT engine runs ONLY the 512
softmax exps [128,512] so its 1.2G elem/s throughput is never wasted):
  - LN on DVE (bn_stats/bn_aggr); gamma/beta broadcast rows; one BATCHED
    xbar-transpose DMA per token tile ([128,C] -> [128, C/128, 128] blocks).
  - Projections: weight-stationary bf16 matmuls, biases applied on DVE
    (tensor_scalar_add), never on ACT.
  - Attention per head pair (even head rows 0-63, odd rows 64-127 of the
    kT chunk -> score matmuls land on disjoint PE row groups and overlap).
    V carries an appended ones column so the attended matmul also emits the
    softmax denominator (psum row 64).  exp on ACT straight out of PSUM
    (scores bounded, no max subtraction).  Scores are emitted one kt ahead
    of the attended matmuls so ACT never starves behind the in-order PE.
  - Warmup: head pair (0,1) q2=0 is interleaved with the context chunks so
    exps begin while K/V projections still run.
  - Out-projection qt tiles are interleaved into the q2=1 attention stream
    (PE has slack there; ACT stays the bottleneck).
"""

import numpy as np

import concourse.bass as bass
import concourse.tile as tile
from concourse import mybir
from concourse.bass_utils import run_bass_kernel_spmd

F32 = mybir.dt.float32
BF16 = mybir.dt.bfloat16
AF = mybir.ActivationFunctionType
OP = mybir.AluOpType

B, NQ_FULL, NK, CQ, CK, H, D = 4, 2048, 2048, 1024, 768, 16, 64
NQ = 1024            # per-core query tokens
N_CORES = 8
EPS = 1e-5
SM_SCALE = 1.0 / np.sqrt(D)  # 0.125

KC_Q = CQ // 128     # 8  contraction chunks for CQ
KC_C = CK // 128     # 6  contraction chunks for CK
NQT = NQ // 128      # 8  query token tiles
NKT = NK // 128      # 16 context token tiles
QC = 512             # psum free-dim limit (fp32)
NQ2 = NQ // QC       # 2


def _split_excess_waits(nc, max_waits=1):
    """walrus in this container accepts at most one sync wait per
    instruction; Tile's kernel-tail drain carries several.  Hoist excess
    waits onto single-wait NOPs that precede the instruction on the same
    engine (absolute sem waits commute, so this is semantics-preserving)."""
    for fn in nc.m.functions:
        for blk in fn.blocks:
            out = []
            dirty = False
            for inst in list(blk.instructions):
                si = inst.sync_info
                if si is not None and len(si.on_wait) > max_waits:
                    waits = list(si.on_wait)
                    for k, w in enumerate(waits[:-max_waits]):
                        nop = mybir.InstNoOp(
                            name=f"wsplit-{inst.name}-{k}", ins=[], outs=[])
                        nop.engine = inst.engine
                        nop.sync_info = mybir.SyncInfo(on_wait=[w], on_update=[])
                        out.append(nop)
                    inst.sync_info = mybir.SyncInfo(
                        on_wait=waits[-max_waits:], on_update=list(si.on_update))
                    dirty = True
                out.append(inst)
            if dirty:
                blk.instructions = out


def _bcast_ap(handle, n_parts, n_free):
    """DRAM [n_free] vector replicated across n_parts partitions."""
    return bass.AP(tensor=handle.ap().tensor, offset=0,
                   ap=[[0, n_parts], [1, n_free]])


def _emit(tc, t, out):
    from contextlib import ExitStack
    nc = tc.nc

    es = ExitStack()
    persist = es.enter_context(tc.tile_pool(name="persist", bufs=1))
    wp = es.enter_context(tc.tile_pool(name="wp", bufs=1))
    wfp = es.enter_context(tc.tile_pool(name="wfp", bufs=2))
    lnp = es.enter_context(tc.tile_pool(name="lnp", bufs=2))
    stp = es.enter_context(tc.tile_pool(name="stp", bufs=4))
    xTp = es.enter_context(tc.tile_pool(name="xTp", bufs=1))
    ep = es.enter_context(tc.tile_pool(name="ep", bufs=2))
    rp = es.enter_context(tc.tile_pool(name="rp", bufs=2))
    scr = es.enter_context(tc.tile_pool(name="scr", bufs=2, space="DRAM"))
    osp = es.enter_context(tc.tile_pool(name="osp", bufs=2))
    pps = es.enter_context(tc.tile_pool(name="pps", bufs=2, space="PSUM"))
    scps = es.enter_context(tc.tile_pool(name="scps", bufs=2, space="PSUM"))
    attps = es.enter_context(tc.tile_pool(name="attps", bufs=2, space="PSUM"))

    # ---- persistent tensors ----
    qTc = [persist.tile([128, NQ], BF16, tag=f"qT{oc}", name=f"qT{oc}")
           for oc in range(KC_Q)]
    kTc = [[persist.tile([128, QC], BF16, tag=f"kT{oc}_{t4}",
                         name=f"kT{oc}_{t4}") for t4 in range(NK // QC)]
           for oc in range(KC_Q)]
    # V with a ones column per head: attended matmul also emits denominator
    v_g = [persist.tile([128, 4, H, D + 1], BF16, tag=f"v{g}", name=f"v{g}")
           for g in range(NKT // 4)]
    attT = persist.tile([128, KC_Q, NQ], BF16, name="attT")
    bq_cols = persist.tile([128, KC_Q], F32)
    bk_cols = persist.tile([128, KC_Q], F32)
    bvb = persist.tile([128, CQ], BF16)
    bob = persist.tile([128, CQ], F32)
    eps_t = persist.tile([128, 1], F32)
    gqb = persist.tile([128, CQ], BF16)
    bqb = persist.tile([128, CQ], BF16)
    gcb = persist.tile([128, CK], BF16)
    bcb = persist.tile([128, CK], BF16)

    nc.vector.memset(eps_t[:, :], EPS)
    nc.scalar.dma_start(out=bq_cols[:, :],
                        in_=t["bq"].ap().rearrange("(j p) -> p j", p=128))
    nc.scalar.dma_start(out=bk_cols[:, :],
                        in_=t["bk"].ap().rearrange("(j p) -> p j", p=128))
    # LN gamma/beta broadcasts first on Pool (SWDGE casts f32->bf16);
    # bvb/bob after — they are needed much later
    nc.gpsimd.dma_start(out=gqb[:, :], in_=_bcast_ap(t["gamma_q"], 128, CQ))
    nc.gpsimd.dma_start(out=bqb[:, :], in_=_bcast_ap(t["beta_q"], 128, CQ))
    nc.gpsimd.dma_start(out=gcb[:, :], in_=_bcast_ap(t["gamma_ctx"], 128, CK))
    nc.gpsimd.dma_start(out=bcb[:, :], in_=_bcast_ap(t["beta_ctx"], 128, CK))
    nc.gpsimd.dma_start(out=bvb[:, :], in_=_bcast_ap(t["bv"], 128, CQ))
    nc.gpsimd.dma_start(out=bob[:, :], in_=_bcast_ap(t["bo"], 128, CQ))

    # ---- weights: fp32 staging -> bf16 on ACT (idle during the ramp;
    # Copy is emitted before any Exp).  wq and wo share slots (tag wA) ----
    def load_w(dram, kc, tag, nm, eng, cvt):
        wf = wfp.tile([128, CQ], F32, tag="wf", name=f"wf_{nm}")
        eng.dma_start(out=wf[:, :], in_=dram.ap()[kc * 128:(kc + 1) * 128, :])
        wb = wp.tile([128, CQ], BF16, tag=tag, bufs=8 if tag == "wA" else 6,
                     name=nm)
        cvt(wb, wf)
        return wb

    def cvt_act(wb, wf):
        nc.scalar.activation(out=wb[:, :], in_=wf[:, :], func=AF.Copy)

    def cvt_dve(wb, wf):
        nc.vector.tensor_copy(out=wb[:, :], in_=wf[:, :])

    wq = [load_w(t["Wq"], kc, "wA", f"wq{kc}",
                 nc.scalar if kc < 4 else nc.sync, cvt_act)
          for kc in range(KC_Q)]
    wk = [load_w(t["Wk"], kc, "wk", f"wk{kc}",
                 nc.scalar if kc < 3 else nc.sync, cvt_act)
          for kc in range(KC_C)]
    wv = [load_w(t["Wv"], kc, "wv", f"wv{kc}",
                 nc.scalar if kc < 3 else nc.sync, cvt_act)
          for kc in range(KC_C)]

    # ---- LN: batch the 4 input loads of a chunk first (the ACT queue
    # head blocks on sqrt->DVE deps, so loads must be issued before) ----
    def ln_load(x_dram, i, C):
        xf = lnp.tile([128, CQ], F32, tag="xf", bufs=3, name=f"xf_{i}_{C}")
        nc.scalar.dma_start(out=xf[:, 0:C],
                            in_=x_dram.ap()[i * 128:(i + 1) * 128, :])
        return xf

    def ln_proc(xf, i, C, n_sub, sub, gb, bb, xT):
        st = stp.tile([128, 3, 6], F32, tag="st", name=f"st_{i}_{C}")
        for s in range(n_sub):
            nc.vector.bn_stats(out=st[:, s, :],
                               in_=xf[:, s * sub:(s + 1) * sub])
        mv = stp.tile([128, 2], F32, tag="mv", name=f"mv_{i}_{C}")
        nc.vector.bn_aggr(out=mv[:, :], in_=st[:, 0:n_sub, :])
        nc.scalar.activation(out=mv[:, 1:2], in_=mv[:, 1:2],
                             func=AF.Sqrt, bias=eps_t[:, :], scale=1.0)
        nc.vector.reciprocal(out=mv[:, 1:2], in_=mv[:, 1:2])
        nc.vector.tensor_scalar(out=xf[:, 0:C], in0=xf[:, 0:C],
                                scalar1=mv[:, 0:1], scalar2=mv[:, 1:2],
                                op0=OP.subtract, op1=OP.mult)
        nc.vector.tensor_mul(out=xf[:, 0:C], in0=xf[:, 0:C], in1=gb[:, :])
        xbf = lnp.tile([128, CQ], BF16, tag="xbf", bufs=1,
                       name=f"xbf_{i}_{C}")
        nc.vector.tensor_add(out=xbf[:, 0:C], in0=xf[:, 0:C], in1=bb[:, :])
        # one xbar transpose for all C/128 channel chunks of this tile
        nc.sync.dma_start(out=xT[:, :, (i % 4) * 128:(i % 4) * 128 + 128],
                          in_=xbf[:, 0:C], transpose=True)

    # ---- query side ----
    for t2 in range(NQ2):
        xTq = xTp.tile([128, KC_Q, QC], BF16, tag="xTq", name=f"xTq_{t2}")
        xfs = [ln_load(t["xq"], t2 * 4 + i, CQ) for i in range(4)]
        for i in range(4):
            ln_proc(xfs[i], t2 * 4 + i, CQ, 2, 512, gqb, bqb, xTq)
        for oc in range(KC_Q):
            ps = pps.tile([128, QC], F32, tag="pp", name=f"psq{oc}_{t2}")
            for kc in range(KC_Q):
                nc.tensor.matmul(ps[:, :],
                                 wq[kc][:, oc * 128:(oc + 1) * 128],
                                 xTq[:, kc, :],
                                 start=(kc == 0), stop=(kc == KC_Q - 1))
            nc.vector.tensor_scalar_add(
                out=qTc[oc][:, t2 * QC:(t2 + 1) * QC], in0=ps[:, :],
                scalar1=bq_cols[:, oc:oc + 1])

    # ---- attention helpers ----
    def attn_scores_kt(hp, q2, kt, sc_pair):
        g, r = kt // 4, kt % 4
        for par in range(2):
            lo = par * 64
            nc.tensor.matmul(
                sc_pair[par][:, :],
                kTc[hp][g][lo:lo + 64, r * 128:(r + 1) * 128],
                qTc[hp][lo:lo + 64, q2 * QC:(q2 + 1) * QC],
                start=True, stop=True)
        e_pair = []
        for par in range(2):
            h = 2 * hp + par
            e = ep.tile([128, QC], BF16, tag=f"e{par}", name=f"e{h}_{q2}_{kt}")
            nc.scalar.activation(out=e[:, :], in_=sc_pair[par][:, :],
                                 func=AF.Exp, scale=SM_SCALE)
            e_pair.append(e)
        return e_pair

    def attn_attend_kt(hp, kt, att_pair, e_pair):
        g, r = kt // 4, kt % 4
        for par in range(2):
            h = 2 * hp + par
            nc.tensor.matmul(att_pair[par][:, :],
                             v_g[g][:, r, h, :],
                             e_pair[par][:, :],
                             start=(kt == 0), stop=(kt == NKT - 1))

    def attn_normalize(hp, q2, att_pair):
        for par in range(2):
            h = 2 * hp + par
            atc = rp.tile([64, QC], F32, tag="atc", name=f"atc{h}_{q2}")
            nc.vector.tensor_copy(out=atc[:, :], in_=att_pair[par][0:D, :])
            # comb row 64 = reciprocal of denominator; rows 0-63 = its
            # partition-broadcast (DRAM bounce) — one tile for both
            comb = rp.tile([65, QC], F32, tag="comb", name=f"comb{h}_{q2}")
            nc.vector.reciprocal(out=comb[64:65, :],
                                 in_=att_pair[par][64:65, :])
            sd = scr.tile([1, QC], F32, tag="sd", name=f"sd{h}_{q2}")
            nc.gpsimd.dma_start(out=sd[:, :], in_=comb[64:65, :])
            nc.gpsimd.dma_start(
                out=comb[0:64, :],
                in_=bass.AP(tensor=sd.tensor, offset=sd.offset,
                            ap=[[0, 64], [1, QC]]))
            if par == 0:
                nc.vector.tensor_mul(
                    out=attT[0:64, hp, q2 * QC:(q2 + 1) * QC],
                    in0=atc[:, :], in1=comb[0:64, :])
            else:
                tm = rp.tile([64, QC], BF16, tag="tm", bufs=1,
                             name=f"tm{h}_{q2}")
                nc.vector.tensor_mul(out=tm[:, :], in0=atc[:, :],
                                     in1=comb[0:64, :])
                nc.sync.dma_start(
                    out=attT[64:128, hp, q2 * QC:(q2 + 1) * QC], in_=tm[:, :])

    def attn_pair_block(hp, q2, kts, att_pair, pending, last_kt):
        """software-pipelined: scores(kt) emitted before attended(kt-1)."""
        for kt in kts:
            sc_pair = [scps.tile([128, QC], F32, tag=f"sc{par}",
                                 name=f"sc{2 * hp + par}_{q2}_{kt}")
                       for par in range(2)]
            e_pair = attn_scores_kt(hp, q2, kt, sc_pair)
            if pending is not None:
                attn_attend_kt(hp, pending[0], att_pair, pending[1])
            pending = (kt, e_pair)
        if kts and kts[-1] == last_kt:
            attn_attend_kt(hp, pending[0], att_pair, pending[1])
            attn_normalize(hp, q2, att_pair)
            pending = None
        return pending

    # warmup attention state: head pair (0,1), q2=0, runs chunk by chunk
    att_w = [attps.tile([D + 1, QC], F32, tag="att", name=f"attw{par}")
             for par in range(2)]
    pend_w = None

    # ---- context side, chunk by chunk, warmup attention interleaved ----
    for t4 in range(NK // QC):
        xTc = xTp.tile([128, KC_C, QC], BF16, tag="xTc", bufs=2,
                       name=f"xTc_{t4}")
        xfs = [ln_load(t["xc"], t4 * 4 + i, CK) for i in range(4)]
        for i in range(4):
            ln_proc(xfs[i], t4 * 4 + i, CK, 3, 256, gcb, bcb, xTc)
        for oc in range(KC_Q):
            ps = pps.tile([128, QC], F32, tag="pp", name=f"psk{oc}_{t4}")
            for kc in range(KC_C):
                nc.tensor.matmul(ps[:, :],
                                 wk[kc][:, oc * 128:(oc + 1) * 128],
                                 xTc[:, kc, :],
                                 start=(kc == 0), stop=(kc == KC_C - 1))
            nc.vector.tensor_scalar_add(out=kTc[oc][t4][:, :], in0=ps[:, :],
                                        scalar1=bk_cols[:, oc:oc + 1])
        for ki in range(4):
            for v2 in range(CQ // QC):
                ps = pps.tile([128, QC], F32, tag="pp", name=f"psv{t4}_{ki}{v2}")
                for kc in range(KC_C):
                    nc.tensor.matmul(ps[:, :],
                                     xTc[:, kc, ki * 128:(ki + 1) * 128],
                                     wv[kc][:, v2 * QC:(v2 + 1) * QC],
                                     start=(kc == 0), stop=(kc == KC_C - 1))
                nc.vector.tensor_tensor(
                    out=v_g[t4][:, ki, v2 * 8:(v2 + 1) * 8, 0:D],
                    in0=ps[:, :].rearrange("p (h d) -> p h d", d=D),
                    in1=bvb[:, v2 * QC:(v2 + 1) * QC].rearrange(
                        "p (h d) -> p h d", d=D),
                    op=OP.add)
            nc.vector.memset(v_g[t4][:, ki, :, D:D + 1], 1.0)
        # warmup: head pair 0 (q2=0) over this chunk's kt range
        pend_w = attn_pair_block(0, 0, list(range(4 * t4, 4 * t4 + 4)),
                                 att_w, pend_w, NKT - 1)

    # wo loads reuse the wq slots (tag wA); sync queue is quiet by now and
    # the ACT queue must stay exp-only, so stage via sync + convert on DVE
    wo = [load_w(t["Wo"], kc, "wA", f"wo{kc}", nc.sync, cvt_dve)
          for kc in range(KC_Q)]

    # ---- out projection helper (interleaved later) ----
    def outproj_qt(qt):
        for cc in range(CQ // QC):
            ps = pps.tile([128, QC], F32, tag="pp", name=f"pso{qt}_{cc}")
            for kc in range(KC_Q):
                nc.tensor.matmul(
                    ps[:, :],
                    attT[:, kc, qt * 128:(qt + 1) * 128],
                    wo[kc][:, cc * QC:(cc + 1) * QC],
                    start=(kc == 0), stop=(kc == KC_Q - 1))
            osb = osp.tile([128, QC], F32, tag="osb", bufs=1,
                           name=f"osb{qt}_{cc}")
            nc.vector.tensor_tensor(out=osb[:, :], in0=ps[:, :],
                                    in1=bob[:, cc * QC:(cc + 1) * QC],
                                    op=OP.add)
            nc.sync.dma_start(
                out=out.ap()[qt * 128:(qt + 1) * 128, cc * QC:(cc + 1) * QC],
                in_=osb[:, :])

    # ---- steady attention: q2=0 pairs 1..7, then q2=1 pairs 0..7 with
    # out-projection qt tiles interleaved into the q2=1 stream ----
    for hp in range(1, H // 2):
        att_pair = [attps.tile([D + 1, QC], F32, tag="att",
                               name=f"att{hp}_0_{par}") for par in range(2)]
        attn_pair_block(hp, 0, list(range(NKT)), att_pair, None, NKT - 1)

    for hp in range(H // 2):
        att_pair = [attps.tile([D + 1, QC], F32, tag="att",
                               name=f"att{hp}_1_{par}") for par in range(2)]
        attn_pair_block(hp, 1, list(range(NKT)), att_pair, None, NKT - 1)
        if hp >= 4:
            outproj_qt(hp - 4)          # qt 0..3 (q2=0 halves) overlap
    for qt in range(4, NQT):
        outproj_qt(qt)

    es.close()


def build():
    nc = bass.Bass("TRN2", target_bir_lowering=False, debug=False,
                   num_devices=N_CORES)
    t = {
        "xq": nc.dram_tensor("xq", [NQ, CQ], F32, kind="ExternalInput"),
        "xc": nc.dram_tensor("xc", [NK, CK], F32, kind="ExternalInput"),
        "Wq": nc.dram_tensor("Wq", [CQ, CQ], F32, kind="ExternalInput"),
        "Wk": nc.dram_tensor("Wk", [CK, CQ], F32, kind="ExternalInput"),
        "Wv": nc.dram_tensor("Wv", [CK, CQ], F32, kind="ExternalInput"),
        "Wo": nc.dram_tensor("Wo", [CQ, CQ], F32, kind="ExternalInput"),
        "bq": nc.dram_tensor("bq", [CQ], F32, kind="ExternalInput"),
        "bk": nc.dram_tensor("bk", [CQ], F32, kind="ExternalInput"),
        "bv": nc.dram_tensor("bv", [CQ], F32, kind="ExternalInput"),
        "bo": nc.dram_tensor("bo", [CQ], F32, kind="ExternalInput"),
        "gamma_q": nc.dram_tensor("gamma_q", [CQ], F32, kind="ExternalInput"),
        "beta_q": nc.dram_tensor("beta_q", [CQ], F32, kind="ExternalInput"),
        "gamma_ctx": nc.dram_tensor("gamma_ctx", [CK], F32, kind="ExternalInput"),
        "beta_ctx": nc.dram_tensor("beta_ctx", [CK], F32, kind="ExternalInput"),
    }
    out = nc.dram_tensor("out", [NQ, CQ], F32, kind="ExternalOutput")
    with tile.TileContext(nc) as tc:
        _emit(tc, t, out)
    _split_excess_waits(nc)
    return nc


_NC = None


def _in_maps(inputs):
    q = np.ascontiguousarray(np.asarray(inputs["query_tokens"], dtype=np.float32))
    c = np.ascontiguousarray(np.asarray(inputs["context_tokens"], dtype=np.float32))
    shared = {k: np.ascontiguousarray(np.asarray(inputs[k], dtype=np.float32))
              for k in ("Wq", "Wk", "Wv", "Wo", "bq", "bk", "bv", "bo",
                        "gamma_q", "beta_q", "gamma_ctx", "beta_ctx")}
    maps = []
    for core in range(N_CORES):
        b, half = core // 2, core % 2
        m = dict(shared)
        m["xq"] = np.ascontiguousarray(q[b, half * NQ:(half + 1) * NQ, :])
        m["xc"] = np.ascontiguousarray(c[b])
        maps.append(m)
    return maps


def run_sharded(inputs, **kwargs):
    global _NC
    if _NC is None:
        _NC = build()
    return run_bass_kernel_spmd(_NC, _in_maps(inputs),
                                core_ids=list(range(N_CORES)), **kwargs)


def kernel(**inputs) -> np.ndarray:
    res = run_sharded(inputs)
    out = np.empty((B, NQ_FULL, CQ), np.float32)
    for core in range(N_CORES):
        b, half = core // 2, core % 2
        out[b, half * NQ:(half + 1) * NQ, :] = res.results[core]["out"]
    return out


# revision 23
# speedup vs baseline: 1.1441x; 1.1441x over previous
"""Trainium2 Bass kernel for CrossAttention (LN -> QKV proj -> MHA -> out proj).

Sharding: data-parallel over (batch, query-half): 8 shards for B=4.
Each core gets a [1024, 1024] query-token slice and the full [2048, 768]
context for its batch, and produces a [1024, 1024] output slice.

v2 design notes (engine-stream oriented; the ACT engine runs ONLY the 512
softmax exps [128,512] so its 1.2G elem/s throughput is never wasted):
  - LN on DVE (bn_stats/bn_aggr); gamma/beta broadcast rows; one BATCHED
    xbar-transpose DMA per token tile ([128,C] -> [128, C/128, 128] blocks).
  - Projections: weight-stationary bf16 matmuls, biases applied on DVE
    (tensor_scalar_add), never on ACT.
  - Attention per head pair (even head rows 0-63, odd rows 64-127 of the
    kT chunk -> score matmuls land on disjoint PE row groups and overlap).
    V carries an appended ones column so the attended matmul also emits the
    softmax denominator (psum row 64).  exp on ACT straight out of PSUM
    (scores bounded, no max subtraction).  Scores are emitted one kt ahead
    of the attended matmuls so ACT never starves behind the in-order PE.
  - Warmup: head pair (0,1) q2=0 is interleaved with the context chunks so
    exps begin while K/V projections still run.
  - Out-projection qt tiles are interleaved into the q2=1 attention stream
    (PE has slack there; ACT stays the bottleneck).
"""

import numpy as np

import concourse.bass as bass
import concourse.tile as tile
from concourse import mybir
from concourse.bass_utils import run_bass_kernel_spmd

F32 = mybir.dt.float32
BF16 = mybir.dt.bfloat16
AF = mybir.ActivationFunctionType
OP = mybir.AluOpType

B, NQ_FULL, NK, CQ, CK, H, D = 4, 2048, 2048, 1024, 768, 16, 64
NQ = 1024            # per-core query tokens
N_CORES = 8
EPS = 1e-5
SM_SCALE = 1.0 / np.sqrt(D)  # 0.125

KC_Q = CQ // 128     # 8  contraction chunks for CQ
KC_C = CK // 128     # 6  contraction chunks for CK
NQT = NQ // 128      # 8  query token tiles
NKT = NK // 128      # 16 context token tiles
QC = 512             # psum free-dim limit (fp32)
NQ2 = NQ // QC       # 2


def _split_excess_waits(nc, max_waits=1):
    """walrus in this container accepts at most one sync wait per
    instruction; Tile's kernel-tail drain carries several.  Hoist excess
    waits onto single-wait NOPs that precede the instruction on the same
    engine (absolute sem waits commute, so this is semantics-preserving)."""
    for fn in nc.m.functions:
        for blk in fn.blocks:
            out = []
            dirty = False
            for inst in list(blk.instructions):
                si = inst.sync_info
                if si is not None and len(si.on_wait) > max_waits:
                    waits = list(si.on_wait)
                    for k, w in enumerate(waits[:-max_waits]):
                        nop = mybir.InstNoOp(
                            name=f"wsplit-{inst.name}-{k}", ins=[], outs=[])
                        nop.engine = inst.engine
                        nop.sync_info = mybir.SyncInfo(on_wait=[w], on_update=[])
                        out.append(nop)
                    inst.sync_info = mybir.SyncInfo(
                        on_wait=waits[-max_waits:], on_update=list(si.on_update))
                    dirty = True
                out.append(inst)
            if dirty:
                blk.instructions = out


def _bcast_ap(handle, n_parts, n_free):
    """DRAM [n_free] vector replicated across n_parts partitions."""
    return bass.AP(tensor=handle.ap().tensor, offset=0,
                   ap=[[0, n_parts], [1, n_free]])


def _emit(tc, t, out):
    from contextlib import ExitStack
    nc = tc.nc

    es = ExitStack()
    persist = es.enter_context(tc.tile_pool(name="persist", bufs=1))
    wp = es.enter_context(tc.tile_pool(name="wp", bufs=1))
    wfp = es.enter_context(tc.tile_pool(name="wfp", bufs=2))
    lnp = es.enter_context(tc.tile_pool(name="lnp", bufs=2))
    stp = es.enter_context(tc.tile_pool(name="stp", bufs=4))
    xTp = es.enter_context(tc.tile_pool(name="xTp", bufs=1))
    ep = es.enter_context(tc.tile_pool(name="ep", bufs=2))
    rp = es.enter_context(tc.tile_pool(name="rp", bufs=2))
    scr = es.enter_context(tc.tile_pool(name="scr", bufs=2, space="DRAM"))
    osp = es.enter_context(tc.tile_pool(name="osp", bufs=2))
    pps = es.enter_context(tc.tile_pool(name="pps", bufs=2, space="PSUM"))
    scps = es.enter_context(tc.tile_pool(name="scps", bufs=2, space="PSUM"))
    attps = es.enter_context(tc.tile_pool(name="attps", bufs=2, space="PSUM"))

    # ---- persistent tensors ----
    qTc = [persist.tile([128, NQ], BF16, tag=f"qT{oc}", name=f"qT{oc}")
           for oc in range(KC_Q)]
    kTc = [[persist.tile([128, QC], BF16, tag=f"kT{oc}_{t4}",
                         name=f"kT{oc}_{t4}") for t4 in range(NK // QC)]
           for oc in range(KC_Q)]
    # V with a ones column per head: attended matmul also emits denominator
    v_g = [persist.tile([128, 4, H, D + 1], BF16, tag=f"v{g}", name=f"v{g}")
           for g in range(NKT // 4)]
    attT = persist.tile([128, KC_Q, NQ], BF16, name="attT")
    bq_cols = persist.tile([128, KC_Q], F32)
    bk_cols = persist.tile([128, KC_Q], F32)
    bvb = persist.tile([128, CQ], BF16)
    bob = persist.tile([128, CQ], F32)
    eps_t = persist.tile([128, 1], F32)
    gqb = persist.tile([128, CQ], BF16)
    bqb = persist.tile([128, CQ], BF16)
    gcb = persist.tile([128, CK], BF16)
    bcb = persist.tile([128, CK], BF16)

    nc.vector.memset(eps_t[:, :], EPS)
    nc.scalar.dma_start(out=bq_cols[:, :],
                        in_=t["bq"].ap().rearrange("(j p) -> p j", p=128))
    nc.scalar.dma_start(out=bk_cols[:, :],
                        in_=t["bk"].ap().rearrange("(j p) -> p j", p=128))
    # LN gamma/beta broadcasts first on Pool (SWDGE casts f32->bf16);
    # bvb/bob after — they are needed much later
    nc.gpsimd.dma_start(out=gqb[:, :], in_=_bcast_ap(t["gamma_q"], 128, CQ))
    nc.gpsimd.dma_start(out=bqb[:, :], in_=_bcast_ap(t["beta_q"], 128, CQ))
    nc.gpsimd.dma_start(out=gcb[:, :], in_=_bcast_ap(t["gamma_ctx"], 128, CK))
    nc.gpsimd.dma_start(out=bcb[:, :], in_=_bcast_ap(t["beta_ctx"], 128, CK))
    nc.gpsimd.dma_start(out=bvb[:, :], in_=_bcast_ap(t["bv"], 128, CQ))
    nc.gpsimd.dma_start(out=bob[:, :], in_=_bcast_ap(t["bo"], 128, CQ))

    # ---- weights: fp32 staging -> bf16 on ACT (idle during the ramp;
    # Copy is emitted before any Exp).  wq and wo share slots (tag wA) ----
    def load_w(dram, kc, tag, nm, eng, cvt):
        wf = wfp.tile([128, CQ], F32, tag="wf", name=f"wf_{nm}")
        eng.dma_start(out=wf[:, :], in_=dram.ap()[kc * 128:(kc + 1) * 128, :])
        wb = wp.tile([128, CQ], BF16, tag=tag, bufs=8 if tag == "wA" else 6,
                     name=nm)
        cvt(wb, wf)
        return wb

    def cvt_gp(wb, wf):
        nc.gpsimd.tensor_copy(out=wb[:, :], in_=wf[:, :])

    def cvt_dve(wb, wf):
        nc.vector.tensor_copy(out=wb[:, :], in_=wf[:, :])

    wq = [load_w(t["Wq"], kc, "wA", f"wq{kc}",
                 nc.scalar if kc < 4 else nc.sync, cvt_gp)
          for kc in range(KC_Q)]
    wk = [load_w(t["Wk"], kc, "wk", f"wk{kc}",
                 nc.scalar if kc < 3 else nc.sync, cvt_gp)
          for kc in range(KC_C)]
    wv = [load_w(t["Wv"], kc, "wv", f"wv{kc}",
                 nc.scalar if kc < 3 else nc.sync, cvt_gp)
          for kc in range(KC_C)]

    # ---- LN one [128, C] token tile -> bf16 -> one batched transpose ----
    def ln_tile(x_dram, i, C, n_sub, sub, gb, bb, xT):
        xf = lnp.tile([128, CQ], F32, tag="xf", name=f"xf_{i}_{C}")
        nc.scalar.dma_start(out=xf[:, 0:C],
                            in_=x_dram.ap()[i * 128:(i + 1) * 128, :])
        st = stp.tile([128, 3, 6], F32, tag="st", name=f"st_{i}_{C}")
        for s in range(n_sub):
            nc.vector.bn_stats(out=st[:, s, :],
                               in_=xf[:, s * sub:(s + 1) * sub])
        mv = stp.tile([128, 2], F32, tag="mv", name=f"mv_{i}_{C}")
        nc.vector.bn_aggr(out=mv[:, :], in_=st[:, 0:n_sub, :])
        nc.scalar.activation(out=mv[:, 1:2], in_=mv[:, 1:2],
                             func=AF.Sqrt, bias=eps_t[:, :], scale=1.0)
        nc.vector.reciprocal(out=mv[:, 1:2], in_=mv[:, 1:2])
        nc.vector.tensor_scalar(out=xf[:, 0:C], in0=xf[:, 0:C],
                                scalar1=mv[:, 0:1], scalar2=mv[:, 1:2],
                                op0=OP.subtract, op1=OP.mult)
        nc.vector.tensor_mul(out=xf[:, 0:C], in0=xf[:, 0:C], in1=gb[:, :])
        xbf = lnp.tile([128, CQ], BF16, tag="xbf", name=f"xbf_{i}_{C}")
        nc.vector.tensor_add(out=xbf[:, 0:C], in0=xf[:, 0:C], in1=bb[:, :])
        # one xbar transpose for all C/128 channel chunks of this tile
        nc.sync.dma_start(out=xT[:, :, (i % 4) * 128:(i % 4) * 128 + 128],
                          in_=xbf[:, 0:C], transpose=True)

    # ---- query side ----
    for t2 in range(NQ2):
        xTq = xTp.tile([128, KC_Q, QC], BF16, tag="xTq", name=f"xTq_{t2}")
        for i in range(4):
            ln_tile(t["xq"], t2 * 4 + i, CQ, 2, 512, gqb, bqb, xTq)
        for oc in range(KC_Q):
            ps = pps.tile([128, QC], F32, tag="pp", name=f"psq{oc}_{t2}")
            for kc in range(KC_Q):
                nc.tensor.matmul(ps[:, :],
                                 wq[kc][:, oc * 128:(oc + 1) * 128],
                                 xTq[:, kc, :],
                                 start=(kc == 0), stop=(kc == KC_Q - 1))
            nc.vector.tensor_scalar_add(
                out=qTc[oc][:, t2 * QC:(t2 + 1) * QC], in0=ps[:, :],
                scalar1=bq_cols[:, oc:oc + 1])

    # ---- attention helpers ----
    def attn_scores_kt(hp, q2, kt, sc_pair):
        g, r = kt // 4, kt % 4
        for par in range(2):
            lo = par * 64
            nc.tensor.matmul(
                sc_pair[par][:, :],
                kTc[hp][g][lo:lo + 64, r * 128:(r + 1) * 128],
                qTc[hp][lo:lo + 64, q2 * QC:(q2 + 1) * QC],
                start=True, stop=True)
        e_pair = []
        for par in range(2):
            h = 2 * hp + par
            e = ep.tile([128, QC], BF16, tag=f"e{par}", name=f"e{h}_{q2}_{kt}")
            nc.scalar.activation(out=e[:, :], in_=sc_pair[par][:, :],
                                 func=AF.Exp, scale=SM_SCALE)
            e_pair.append(e)
        return e_pair

    def attn_attend_kt(hp, kt, att_pair, e_pair):
        g, r = kt // 4, kt % 4
        for par in range(2):
            h = 2 * hp + par
            nc.tensor.matmul(att_pair[par][:, :],
                             v_g[g][:, r, h, :],
                             e_pair[par][:, :],
                             start=(kt == 0), stop=(kt == NKT - 1))

    def attn_normalize(hp, q2, att_pair):
        for par in range(2):
            h = 2 * hp + par
            atc = rp.tile([64, QC], F32, tag="atc", name=f"atc{h}_{q2}")
            nc.vector.tensor_copy(out=atc[:, :], in_=att_pair[par][0:D, :])
            # comb row 64 = reciprocal of denominator; rows 0-63 = its
            # partition-broadcast (DRAM bounce) — one tile for both
            comb = rp.tile([65, QC], F32, tag="comb", name=f"comb{h}_{q2}")
            nc.vector.reciprocal(out=comb[64:65, :],
                                 in_=att_pair[par][64:65, :])
            sd = scr.tile([1, QC], F32, tag="sd", name=f"sd{h}_{q2}")
            nc.gpsimd.dma_start(out=sd[:, :], in_=comb[64:65, :])
            nc.gpsimd.dma_start(
                out=comb[0:64, :],
                in_=bass.AP(tensor=sd.tensor, offset=sd.offset,
                            ap=[[0, 64], [1, QC]]))
            if par == 0:
                nc.vector.tensor_mul(
                    out=attT[0:64, hp, q2 * QC:(q2 + 1) * QC],
                    in0=atc[:, :], in1=comb[0:64, :])
            else:
                tm = rp.tile([64, QC], BF16, tag="tm", bufs=1,
                             name=f"tm{h}_{q2}")
                nc.vector.tensor_mul(out=tm[:, :], in0=atc[:, :],
                                     in1=comb[0:64, :])
                nc.sync.dma_start(
                    out=attT[64:128, hp, q2 * QC:(q2 + 1) * QC], in_=tm[:, :])

    def attn_pair_block(hp, q2, kts, att_pair, pending, last_kt):
        """software-pipelined: scores(kt) emitted before attended(kt-1)."""
        for kt in kts:
            sc_pair = [scps.tile([128, QC], F32, tag=f"sc{par}",
                                 name=f"sc{2 * hp + par}_{q2}_{kt}")
                       for par in range(2)]
            e_pair = attn_scores_kt(hp, q2, kt, sc_pair)
            if pending is not None:
                attn_attend_kt(hp, pending[0], att_pair, pending[1])
            pending = (kt, e_pair)
        if kts and kts[-1] == last_kt:
            attn_attend_kt(hp, pending[0], att_pair, pending[1])
            attn_normalize(hp, q2, att_pair)
            pending = None
        return pending

    # warmup attention state: head pair (0,1), q2=0, runs chunk by chunk
    att_w = [attps.tile([D + 1, QC], F32, tag="att", name=f"attw{par}")
             for par in range(2)]
    pend_w = None

    # ---- context side, chunk by chunk, warmup attention interleaved ----
    for t4 in range(NK // QC):
        xTc = xTp.tile([128, KC_C, QC], BF16, tag="xTc", bufs=2,
                       name=f"xTc_{t4}")
        for i in range(4):
            ln_tile(t["xc"], t4 * 4 + i, CK, 3, 256, gcb, bcb, xTc)
        for oc in range(KC_Q):
            ps = pps.tile([128, QC], F32, tag="pp", name=f"psk{oc}_{t4}")
            for kc in range(KC_C):
                nc.tensor.matmul(ps[:, :],
                                 wk[kc][:, oc * 128:(oc + 1) * 128],
                                 xTc[:, kc, :],
                                 start=(kc == 0), stop=(kc == KC_C - 1))
            nc.vector.tensor_scalar_add(out=kTc[oc][t4][:, :], in0=ps[:, :],
                                        scalar1=bk_cols[:, oc:oc + 1])
        for ki in range(4):
            for v2 in range(CQ // QC):
                ps = pps.tile([128, QC], F32, tag="pp", name=f"psv{t4}_{ki}{v2}")
                for kc in range(KC_C):
                    nc.tensor.matmul(ps[:, :],
                                     xTc[:, kc, ki * 128:(ki + 1) * 128],
                                     wv[kc][:, v2 * QC:(v2 + 1) * QC],
                                     start=(kc == 0), stop=(kc == KC_C - 1))
                nc.vector.tensor_tensor(
                    out=v_g[t4][:, ki, v2 * 8:(v2 + 1) * 8, 0:D],
                    in0=ps[:, :].rearrange("p (h d) -> p h d", d=D),
                    in1=bvb[:, v2 * QC:(v2 + 1) * QC].rearrange(
                        "p (h d) -> p h d", d=D),
                    op=OP.add)
            nc.vector.memset(v_g[t4][:, ki, :, D:D + 1], 1.0)
        # warmup: head pair 0 (q2=0) over this chunk's kt range
        pend_w = attn_pair_block(0, 0, list(range(4 * t4, 4 * t4 + 4)),
                                 att_w, pend_w, NKT - 1)

    # wo loads reuse the wq slots (tag wA); sync queue is quiet by now and
    # the ACT queue must stay exp-only, so stage via sync + convert on DVE
    wo = [load_w(t["Wo"], kc, "wA", f"wo{kc}", nc.sync, cvt_dve)
          for kc in range(KC_Q)]

    # ---- out projection helper (interleaved later) ----
    def outproj_qt(qt):
        for cc in range(CQ // QC):
            ps = pps.tile([128, QC], F32, tag="pp", name=f"pso{qt}_{cc}")
            for kc in range(KC_Q):
                nc.tensor.matmul(
                    ps[:, :],
                    attT[:, kc, qt * 128:(qt + 1) * 128],
                    wo[kc][:, cc * QC:(cc + 1) * QC],
                    start=(kc == 0), stop=(kc == KC_Q - 1))
            osb = osp.tile([128, QC], F32, tag="osb", bufs=1,
                           name=f"osb{qt}_{cc}")
            nc.vector.tensor_tensor(out=osb[:, :], in0=ps[:, :],
                                    in1=bob[:, cc * QC:(cc + 1) * QC],
                                    op=OP.add)
            nc.sync.dma_start(
                out=out.ap()[qt * 128:(qt + 1) * 128, cc * QC:(cc + 1) * QC],
                in_=osb[:, :])

    # ---- steady attention: q2=0 pairs 1..7, then q2=1 pairs 0..7 with
    # out-projection qt tiles interleaved into the q2=1 stream ----
    for hp in range(1, H // 2):
        att_pair = [attps.tile([D + 1, QC], F32, tag="att",
                               name=f"att{hp}_0_{par}") for par in range(2)]
        attn_pair_block(hp, 0, list(range(NKT)), att_pair, None, NKT - 1)

    for hp in range(H // 2):
        att_pair = [attps.tile([D + 1, QC], F32, tag="att",
                               name=f"att{hp}_1_{par}") for par in range(2)]
        attn_pair_block(hp, 1, list(range(NKT)), att_pair, None, NKT - 1)
        if hp >= 4:
            outproj_qt(hp - 4)          # qt 0..3 (q2=0 halves) overlap
    for qt in range(4, NQT):
        outproj_qt(qt)

    es.close()


def build():
    nc = bass.Bass("TRN2", target_bir_lowering=False, debug=False,
                   num_devices=N_CORES)
    t = {
        "xq": nc.dram_tensor("xq", [NQ, CQ], F32, kind="ExternalInput"),
        "xc": nc.dram_tensor("xc", [NK, CK], F32, kind="ExternalInput"),
        "Wq": nc.dram_tensor("Wq", [CQ, CQ], F32, kind="ExternalInput"),
        "Wk": nc.dram_tensor("Wk", [CK, CQ], F32, kind="ExternalInput"),
        "Wv": nc.dram_tensor("Wv", [CK, CQ], F32, kind="ExternalInput"),
        "Wo": nc.dram_tensor("Wo", [CQ, CQ], F32, kind="ExternalInput"),
        "bq": nc.dram_tensor("bq", [CQ], F32, kind="ExternalInput"),
        "bk": nc.dram_tensor("bk", [CQ], F32, kind="ExternalInput"),
        "bv": nc.dram_tensor("bv", [CQ], F32, kind="ExternalInput"),
        "bo": nc.dram_tensor("bo", [CQ], F32, kind="ExternalInput"),
        "gamma_q": nc.dram_tensor("gamma_q", [CQ], F32, kind="ExternalInput"),
        "beta_q": nc.dram_tensor("beta_q", [CQ], F32, kind="ExternalInput"),
        "gamma_ctx": nc.dram_tensor("gamma_ctx", [CK], F32, kind="ExternalInput"),
        "beta_ctx": nc.dram_tensor("beta_ctx", [CK], F32, kind="ExternalInput"),
    }
    out = nc.dram_tensor("out", [NQ, CQ], F32, kind="ExternalOutput")
    with tile.TileContext(nc) as tc:
        _emit(tc, t, out)
    _split_excess_waits(nc)
    return nc


_NC = None


def _in_maps(inputs):
    q = np.ascontiguousarray(np.asarray(inputs["query_tokens"], dtype=np.float32))
    c = np.ascontiguousarray(np.asarray(inputs["context_tokens"], dtype=np.float32))
    shared = {k: np.ascontiguousarray(np.asarray(inputs[k], dtype=np.float32))
              for k in ("Wq", "Wk", "Wv", "Wo", "bq", "bk", "bv", "bo",
                        "gamma_q", "beta_q", "gamma_ctx", "beta_ctx")}
    maps = []
    for core in range(N_CORES):
        b, half = core // 2, core % 2
        m = dict(shared)
        m["xq"] = np.ascontiguousarray(q[b, half * NQ:(half + 1) * NQ, :])
        m["xc"] = np.ascontiguousarray(c[b])
        maps.append(m)
    return maps


def run_sharded(inputs, **kwargs):
    global _NC
    if _NC is None:
        _NC = build()
    return run_bass_kernel_spmd(_NC, _in_maps(inputs),
                                core_ids=list(range(N_CORES)), **kwargs)


def kernel(**inputs) -> np.ndarray:
    res = run_sharded(inputs)
    out = np.empty((B, NQ_FULL, CQ), np.float32)
    for core in range(N_CORES):
        b, half = core // 2, core % 2
        out[b, half * NQ:(half + 1) * NQ, :] = res.results[core]["out"]
    return out


# revision 24
# speedup vs baseline: 1.2530x; 1.0952x over previous
"""Trainium2 Bass kernel for CrossAttention (LN -> QKV proj -> MHA -> out proj).

Sharding: data-parallel over (batch, query-half): 8 shards for B=4.
Each core gets a [1024, 1024] query-token slice and the full [2048, 768]
context for its batch, and produces a [1024, 1024] output slice.

v2 design notes (engine-stream oriented; the ACT engine runs ONLY the 512
softmax exps [128,512] so its 1.2G elem/s throughput is never wasted):
  - LN on DVE (bn_stats/bn_aggr); gamma/beta broadcast rows; one BATCHED
    xbar-transpose DMA per token tile ([128,C] -> [128, C/128, 128] blocks).
  - Projections: weight-stationary bf16 matmuls, biases applied on DVE
    (tensor_scalar_add), never on ACT.
  - Attention per head pair (even head rows 0-63, odd rows 64-127 of the
    kT chunk -> score matmuls land on disjoint PE row groups and overlap).
    V carries an appended ones column so the attended matmul also emits the
    softmax denominator (psum row 64).  exp on ACT straight out of PSUM
    (scores bounded, no max subtraction).  Scores are emitted one kt ahead
    of the attended matmuls so ACT never starves behind the in-order PE.
  - Warmup: head pair (0,1) q2=0 is interleaved with the context chunks so
    exps begin while K/V projections still run.
  - Out-projection qt tiles are interleaved into the q2=1 attention stream
    (PE has slack there; ACT stays the bottleneck).
"""

import numpy as np

import concourse.bass as bass
import concourse.tile as tile
from concourse import mybir
from concourse.bass_utils import run_bass_kernel_spmd

F32 = mybir.dt.float32
BF16 = mybir.dt.bfloat16
AF = mybir.ActivationFunctionType
OP = mybir.AluOpType

B, NQ_FULL, NK, CQ, CK, H, D = 4, 2048, 2048, 1024, 768, 16, 64
NQ = 1024            # per-core query tokens
N_CORES = 8
EPS = 1e-5
SM_SCALE = 1.0 / np.sqrt(D)  # 0.125

KC_Q = CQ // 128     # 8  contraction chunks for CQ
KC_C = CK // 128     # 6  contraction chunks for CK
NQT = NQ // 128      # 8  query token tiles
NKT = NK // 128      # 16 context token tiles
QC = 512             # psum free-dim limit (fp32)
NQ2 = NQ // QC       # 2


def _split_excess_waits(nc, max_waits=1):
    """walrus in this container accepts at most one sync wait per
    instruction; Tile's kernel-tail drain carries several.  Hoist excess
    waits onto single-wait NOPs that precede the instruction on the same
    engine (absolute sem waits commute, so this is semantics-preserving)."""
    for fn in nc.m.functions:
        for blk in fn.blocks:
            out = []
            dirty = False
            for inst in list(blk.instructions):
                si = inst.sync_info
                if si is not None and len(si.on_wait) > max_waits:
                    waits = list(si.on_wait)
                    for k, w in enumerate(waits[:-max_waits]):
                        nop = mybir.InstNoOp(
                            name=f"wsplit-{inst.name}-{k}", ins=[], outs=[])
                        nop.engine = inst.engine
                        nop.sync_info = mybir.SyncInfo(on_wait=[w], on_update=[])
                        out.append(nop)
                    inst.sync_info = mybir.SyncInfo(
                        on_wait=waits[-max_waits:], on_update=list(si.on_update))
                    dirty = True
                out.append(inst)
            if dirty:
                blk.instructions = out


def _bcast_ap(handle, n_parts, n_free):
    """DRAM [n_free] vector replicated across n_parts partitions."""
    return bass.AP(tensor=handle.ap().tensor, offset=0,
                   ap=[[0, n_parts], [1, n_free]])


def _emit(tc, t, out):
    from contextlib import ExitStack
    nc = tc.nc

    es = ExitStack()
    persist = es.enter_context(tc.tile_pool(name="persist", bufs=1))
    wp = es.enter_context(tc.tile_pool(name="wp", bufs=1))
    wfp = es.enter_context(tc.tile_pool(name="wfp", bufs=2))
    lnp = es.enter_context(tc.tile_pool(name="lnp", bufs=2))
    stp = es.enter_context(tc.tile_pool(name="stp", bufs=4))
    xTp = es.enter_context(tc.tile_pool(name="xTp", bufs=1))
    ep = es.enter_context(tc.tile_pool(name="ep", bufs=2))
    rp = es.enter_context(tc.tile_pool(name="rp", bufs=2))
    scr = es.enter_context(tc.tile_pool(name="scr", bufs=2, space="DRAM"))
    osp = es.enter_context(tc.tile_pool(name="osp", bufs=2))
    pps = es.enter_context(tc.tile_pool(name="pps", bufs=2, space="PSUM"))
    scps = es.enter_context(tc.tile_pool(name="scps", bufs=2, space="PSUM"))
    attps = es.enter_context(tc.tile_pool(name="attps", bufs=2, space="PSUM"))

    # ---- persistent tensors ----
    qTc = [persist.tile([128, NQ], BF16, tag=f"qT{oc}", name=f"qT{oc}")
           for oc in range(KC_Q)]
    kTc = [[persist.tile([128, QC], BF16, tag=f"kT{oc}_{t4}",
                         name=f"kT{oc}_{t4}") for t4 in range(NK // QC)]
           for oc in range(KC_Q)]
    # V with a ones column per head: attended matmul also emits denominator
    v_g = [persist.tile([128, 4, H, D + 1], BF16, tag=f"v{g}", name=f"v{g}")
           for g in range(NKT // 4)]
    attT = persist.tile([128, KC_Q, NQ], BF16, name="attT")
    bq_cols = persist.tile([128, KC_Q], F32)
    bk_cols = persist.tile([128, KC_Q], F32)
    bvb = persist.tile([128, CQ], BF16)
    bob = persist.tile([128, CQ], F32)
    eps_t = persist.tile([128, 1], F32)
    gq_cols = persist.tile([128, KC_Q], F32)
    bqg_cols = persist.tile([128, KC_Q], F32)
    gc_cols = persist.tile([128, KC_C], F32)
    bcg_cols = persist.tile([128, KC_C], F32)

    nc.vector.memset(eps_t[:, :], EPS)
    nc.scalar.dma_start(out=bq_cols[:, :],
                        in_=t["bq"].ap().rearrange("(j p) -> p j", p=128))
    nc.scalar.dma_start(out=bk_cols[:, :],
                        in_=t["bk"].ap().rearrange("(j p) -> p j", p=128))
    # gamma/beta as per-chunk per-partition columns (applied after the
    # transpose, where channel is the partition axis)
    nc.scalar.dma_start(out=gq_cols[:, :],
                        in_=t["gamma_q"].ap().rearrange("(j p) -> p j", p=128))
    nc.scalar.dma_start(out=bqg_cols[:, :],
                        in_=t["beta_q"].ap().rearrange("(j p) -> p j", p=128))
    nc.scalar.dma_start(out=gc_cols[:, :],
                        in_=t["gamma_ctx"].ap().rearrange("(j p) -> p j", p=128))
    nc.scalar.dma_start(out=bcg_cols[:, :],
                        in_=t["beta_ctx"].ap().rearrange("(j p) -> p j", p=128))
    nc.gpsimd.dma_start(out=bvb[:, :], in_=_bcast_ap(t["bv"], 128, CQ))
    nc.gpsimd.dma_start(out=bob[:, :], in_=_bcast_ap(t["bo"], 128, CQ))

    # ---- weights: fp32 staging -> bf16 on ACT (idle during the ramp;
    # Copy is emitted before any Exp).  wq and wo share slots (tag wA) ----
    def load_w(dram, kc, tag, nm, eng, cvt):
        wf = wfp.tile([128, CQ], F32, tag="wf", name=f"wf_{nm}")
        eng.dma_start(out=wf[:, :], in_=dram.ap()[kc * 128:(kc + 1) * 128, :])
        wb = wp.tile([128, CQ], BF16, tag=tag, bufs=8 if tag == "wA" else 6,
                     name=nm)
        cvt(wb, wf)
        return wb

    def cvt_gp(wb, wf):
        nc.gpsimd.tensor_copy(out=wb[:, :], in_=wf[:, :])

    def cvt_dve(wb, wf):
        nc.vector.tensor_copy(out=wb[:, :], in_=wf[:, :])

    wq = [load_w(t["Wq"], kc, "wA", f"wq{kc}",
                 nc.scalar if kc < 4 else nc.sync, cvt_gp)
          for kc in range(KC_Q)]
    wk = [load_w(t["Wk"], kc, "wk", f"wk{kc}",
                 nc.scalar if kc < 3 else nc.sync, cvt_gp)
          for kc in range(KC_C)]
    wv = [load_w(t["Wv"], kc, "wv", f"wv{kc}",
                 nc.scalar if kc < 3 else nc.sync, cvt_gp)
          for kc in range(KC_C)]

    # ---- LN one [128, C] token tile -> bf16 -> one batched transpose;
    # gamma/beta are applied post-transpose as per-partition scalars ----
    def ln_tile(x_dram, i, C, n_sub, sub, xT):
        xf = lnp.tile([128, CQ], F32, tag="xf", name=f"xf_{i}_{C}")
        nc.scalar.dma_start(out=xf[:, 0:C],
                            in_=x_dram.ap()[i * 128:(i + 1) * 128, :])
        st = stp.tile([128, 3, 6], F32, tag="st", name=f"st_{i}_{C}")
        for s in range(n_sub):
            nc.vector.bn_stats(out=st[:, s, :],
                               in_=xf[:, s * sub:(s + 1) * sub])
        mv = stp.tile([128, 2], F32, tag="mv", name=f"mv_{i}_{C}")
        nc.vector.bn_aggr(out=mv[:, :], in_=st[:, 0:n_sub, :])
        nc.scalar.activation(out=mv[:, 1:2], in_=mv[:, 1:2],
                             func=AF.Sqrt, bias=eps_t[:, :], scale=1.0)
        nc.vector.reciprocal(out=mv[:, 1:2], in_=mv[:, 1:2])
        xbf = lnp.tile([128, CQ], BF16, tag="xbf", name=f"xbf_{i}_{C}")
        nc.vector.tensor_scalar(out=xbf[:, 0:C], in0=xf[:, 0:C],
                                scalar1=mv[:, 0:1], scalar2=mv[:, 1:2],
                                op0=OP.subtract, op1=OP.mult)
        # one xbar transpose for all C/128 channel chunks of this tile
        nc.sync.dma_start(out=xT[:, :, (i % 4) * 128:(i % 4) * 128 + 128],
                          in_=xbf[:, 0:C], transpose=True)

    # ---- query side (t2=1 deferred into the ctx loop: warmup only
    # needs tokens 0-511) ----
    def q_chunk(t2):
        xTq = xTp.tile([128, KC_Q, QC], BF16, tag="xTq", name=f"xTq_{t2}")
        for i in range(4):
            ln_tile(t["xq"], t2 * 4 + i, CQ, 2, 512, xTq)
        for kc in range(KC_Q):
            nc.vector.tensor_scalar(out=xTq[:, kc, :], in0=xTq[:, kc, :],
                                    scalar1=gq_cols[:, kc:kc + 1],
                                    scalar2=bqg_cols[:, kc:kc + 1],
                                    op0=OP.mult, op1=OP.add)
        for oc in range(KC_Q):
            ps = pps.tile([128, QC], F32, tag="pp", name=f"psq{oc}_{t2}")
            for kc in range(KC_Q):
                nc.tensor.matmul(ps[:, :],
                                 wq[kc][:, oc * 128:(oc + 1) * 128],
                                 xTq[:, kc, :],
                                 start=(kc == 0), stop=(kc == KC_Q - 1))
            nc.vector.tensor_scalar_add(
                out=qTc[oc][:, t2 * QC:(t2 + 1) * QC], in0=ps[:, :],
                scalar1=bq_cols[:, oc:oc + 1])

    q_chunk(0)

    # ---- attention helpers ----
    def attn_scores_kt(hp, q2, kt, sc_pair):
        g, r = kt // 4, kt % 4
        for par in range(2):
            lo = par * 64
            nc.tensor.matmul(
                sc_pair[par][:, :],
                kTc[hp][g][lo:lo + 64, r * 128:(r + 1) * 128],
                qTc[hp][lo:lo + 64, q2 * QC:(q2 + 1) * QC],
                start=True, stop=True)
        e_pair = []
        for par in range(2):
            h = 2 * hp + par
            e = ep.tile([128, QC], BF16, tag=f"e{par}", name=f"e{h}_{q2}_{kt}")
            nc.scalar.activation(out=e[:, :], in_=sc_pair[par][:, :],
                                 func=AF.Exp, scale=SM_SCALE)
            e_pair.append(e)
        return e_pair

    def attn_attend_kt(hp, kt, att_pair, e_pair):
        g, r = kt // 4, kt % 4
        for par in range(2):
            h = 2 * hp + par
            nc.tensor.matmul(att_pair[par][:, :],
                             v_g[g][:, r, h, :],
                             e_pair[par][:, :],
                             start=(kt == 0), stop=(kt == NKT - 1))

    def attn_normalize(hp, q2, att_pair):
        for par in range(2):
            h = 2 * hp + par
            atc = rp.tile([64, QC], F32, tag="atc", name=f"atc{h}_{q2}")
            nc.vector.tensor_copy(out=atc[:, :], in_=att_pair[par][0:D, :])
            # comb row 64 = reciprocal of denominator; rows 0-63 = its
            # partition-broadcast (DRAM bounce) — one tile for both
            comb = rp.tile([65, QC], F32, tag="comb", name=f"comb{h}_{q2}")
            nc.vector.reciprocal(out=comb[64:65, :],
                                 in_=att_pair[par][64:65, :])
            sd = scr.tile([1, QC], F32, tag="sd", name=f"sd{h}_{q2}")
            nc.gpsimd.dma_start(out=sd[:, :], in_=comb[64:65, :])
            nc.gpsimd.dma_start(
                out=comb[0:64, :],
                in_=bass.AP(tensor=sd.tensor, offset=sd.offset,
                            ap=[[0, 64], [1, QC]]))
            if par == 0:
                nc.vector.tensor_mul(
                    out=attT[0:64, hp, q2 * QC:(q2 + 1) * QC],
                    in0=atc[:, :], in1=comb[0:64, :])
            else:
                tm = rp.tile([64, QC], BF16, tag="tm", bufs=1,
                             name=f"tm{h}_{q2}")
                nc.vector.tensor_mul(out=tm[:, :], in0=atc[:, :],
                                     in1=comb[0:64, :])
                nc.sync.dma_start(
                    out=attT[64:128, hp, q2 * QC:(q2 + 1) * QC], in_=tm[:, :])

    def attn_pair_block(hp, q2, kts, att_pair, pending, last_kt):
        """software-pipelined: scores(kt) emitted before attended(kt-1)."""
        for kt in kts:
            sc_pair = [scps.tile([128, QC], F32, tag=f"sc{par}",
                                 name=f"sc{2 * hp + par}_{q2}_{kt}")
                       for par in range(2)]
            e_pair = attn_scores_kt(hp, q2, kt, sc_pair)
            if pending is not None:
                attn_attend_kt(hp, pending[0], att_pair, pending[1])
            pending = (kt, e_pair)
        if kts and kts[-1] == last_kt:
            attn_attend_kt(hp, pending[0], att_pair, pending[1])
            attn_normalize(hp, q2, att_pair)
            pending = None
        return pending

    # warmup attention state: head pair (0,1), q2=0, runs chunk by chunk
    att_w = [attps.tile([D + 1, QC], F32, tag="att", name=f"attw{par}")
             for par in range(2)]
    pend_w = None

    # ---- context side, chunk by chunk, warmup attention interleaved ----
    for t4 in range(NK // QC):
        xTc = xTp.tile([128, KC_C, QC], BF16, tag="xTc", bufs=2,
                       name=f"xTc_{t4}")
        for i in range(4):
            ln_tile(t["xc"], t4 * 4 + i, CK, 3, 256, xTc)
        for kc in range(KC_C):
            nc.vector.tensor_scalar(out=xTc[:, kc, :], in0=xTc[:, kc, :],
                                    scalar1=gc_cols[:, kc:kc + 1],
                                    scalar2=bcg_cols[:, kc:kc + 1],
                                    op0=OP.mult, op1=OP.add)
        for oc in range(KC_Q):
            ps = pps.tile([128, QC], F32, tag="pp", name=f"psk{oc}_{t4}")
            for kc in range(KC_C):
                nc.tensor.matmul(ps[:, :],
                                 wk[kc][:, oc * 128:(oc + 1) * 128],
                                 xTc[:, kc, :],
                                 start=(kc == 0), stop=(kc == KC_C - 1))
            nc.vector.tensor_scalar_add(out=kTc[oc][t4][:, :], in0=ps[:, :],
                                        scalar1=bk_cols[:, oc:oc + 1])
        for ki in range(4):
            for v2 in range(CQ // QC):
                ps = pps.tile([128, QC], F32, tag="pp", name=f"psv{t4}_{ki}{v2}")
                for kc in range(KC_C):
                    nc.tensor.matmul(ps[:, :],
                                     xTc[:, kc, ki * 128:(ki + 1) * 128],
                                     wv[kc][:, v2 * QC:(v2 + 1) * QC],
                                     start=(kc == 0), stop=(kc == KC_C - 1))
                nc.vector.tensor_tensor(
                    out=v_g[t4][:, ki, v2 * 8:(v2 + 1) * 8, 0:D],
                    in0=ps[:, :].rearrange("p (h d) -> p h d", d=D),
                    in1=bvb[:, v2 * QC:(v2 + 1) * QC].rearrange(
                        "p (h d) -> p h d", d=D),
                    op=OP.add)
            nc.vector.memset(v_g[t4][:, ki, :, D:D + 1], 1.0)
        # warmup: head pair 0 (q2=0) over this chunk's kt range
        pend_w = attn_pair_block(0, 0, list(range(4 * t4, 4 * t4 + 4)),
                                 att_w, pend_w, NKT - 1)
        if t4 == 0:
            q_chunk(1)

    # wo loads reuse the wq slots (tag wA); sync queue is quiet by now and
    # the ACT queue must stay exp-only, so stage via sync + convert on DVE
    wo = [load_w(t["Wo"], kc, "wA", f"wo{kc}", nc.sync, cvt_dve)
          for kc in range(KC_Q)]

    # ---- out projection helper (interleaved later) ----
    def outproj_qt(qt):
        for cc in range(CQ // QC):
            ps = pps.tile([128, QC], F32, tag="pp", name=f"pso{qt}_{cc}")
            for kc in range(KC_Q):
                nc.tensor.matmul(
                    ps[:, :],
                    attT[:, kc, qt * 128:(qt + 1) * 128],
                    wo[kc][:, cc * QC:(cc + 1) * QC],
                    start=(kc == 0), stop=(kc == KC_Q - 1))
            osb = osp.tile([128, QC], F32, tag="osb", bufs=1,
                           name=f"osb{qt}_{cc}")
            nc.vector.tensor_tensor(out=osb[:, :], in0=ps[:, :],
                                    in1=bob[:, cc * QC:(cc + 1) * QC],
                                    op=OP.add)
            nc.sync.dma_start(
                out=out.ap()[qt * 128:(qt + 1) * 128, cc * QC:(cc + 1) * QC],
                in_=osb[:, :])

    # ---- steady attention: q2=0 pairs 1..7, then q2=1 pairs 0..7 with
    # out-projection qt tiles interleaved into the q2=1 stream ----
    for hp in range(1, H // 2):
        att_pair = [attps.tile([D + 1, QC], F32, tag="att",
                               name=f"att{hp}_0_{par}") for par in range(2)]
        attn_pair_block(hp, 0, list(range(NKT)), att_pair, None, NKT - 1)

    for hp in range(H // 2):
        att_pair = [attps.tile([D + 1, QC], F32, tag="att",
                               name=f"att{hp}_1_{par}") for par in range(2)]
        attn_pair_block(hp, 1, list(range(NKT)), att_pair, None, NKT - 1)
        if hp >= 4:
            outproj_qt(hp - 4)          # qt 0..3 (q2=0 halves) overlap
    for qt in range(4, NQT):
        outproj_qt(qt)

    es.close()


def build():
    nc = bass.Bass("TRN2", target_bir_lowering=False, debug=False,
                   num_devices=N_CORES)
    t = {
        "xq": nc.dram_tensor("xq", [NQ, CQ], F32, kind="ExternalInput"),
        "xc": nc.dram_tensor("xc", [NK, CK], F32, kind="ExternalInput"),
        "Wq": nc.dram_tensor("Wq", [CQ, CQ], F32, kind="ExternalInput"),
        "Wk": nc.dram_tensor("Wk", [CK, CQ], F32, kind="ExternalInput"),
        "Wv": nc.dram_tensor("Wv", [CK, CQ], F32, kind="ExternalInput"),
        "Wo": nc.dram_tensor("Wo", [CQ, CQ], F32, kind="ExternalInput"),
        "bq": nc.dram_tensor("bq", [CQ], F32, kind="ExternalInput"),
        "bk": nc.dram_tensor("bk", [CQ], F32, kind="ExternalInput"),
        "bv": nc.dram_tensor("bv", [CQ], F32, kind="ExternalInput"),
        "bo": nc.dram_tensor("bo", [CQ], F32, kind="ExternalInput"),
        "gamma_q": nc.dram_tensor("gamma_q", [CQ], F32, kind="ExternalInput"),
        "beta_q": nc.dram_tensor("beta_q", [CQ], F32, kind="ExternalInput"),
        "gamma_ctx": nc.dram_tensor("gamma_ctx", [CK], F32, kind="ExternalInput"),
        "beta_ctx": nc.dram_tensor("beta_ctx", [CK], F32, kind="ExternalInput"),
    }
    out = nc.dram_tensor("out", [NQ, CQ], F32, kind="ExternalOutput")
    with tile.TileContext(nc) as tc:
        _emit(tc, t, out)
    _split_excess_waits(nc)
    return nc


_NC = None


def _in_maps(inputs):
    q = np.ascontiguousarray(np.asarray(inputs["query_tokens"], dtype=np.float32))
    c = np.ascontiguousarray(np.asarray(inputs["context_tokens"], dtype=np.float32))
    shared = {k: np.ascontiguousarray(np.asarray(inputs[k], dtype=np.float32))
              for k in ("Wq", "Wk", "Wv", "Wo", "bq", "bk", "bv", "bo",
                        "gamma_q", "beta_q", "gamma_ctx", "beta_ctx")}
    maps = []
    for core in range(N_CORES):
        b, half = core // 2, core % 2
        m = dict(shared)
        m["xq"] = np.ascontiguousarray(q[b, half * NQ:(half + 1) * NQ, :])
        m["xc"] = np.ascontiguousarray(c[b])
        maps.append(m)
    return maps


def run_sharded(inputs, **kwargs):
    global _NC
    if _NC is None:
        _NC = build()
    return run_bass_kernel_spmd(_NC, _in_maps(inputs),
                                core_ids=list(range(N_CORES)), **kwargs)


def kernel(**inputs) -> np.ndarray:
    res = run_sharded(inputs)
    out = np.empty((B, NQ_FULL, CQ), np.float32)
    for core in range(N_CORES):
        b, half = core // 2, core % 2
        out[b, half * NQ:(half + 1) * NQ, :] = res.results[core]["out"]
    return out


# revision 26
# speedup vs baseline: 8.1073x; 6.4704x over previous
"""Trainium2 Bass kernel for CrossAttention (LN -> QKV proj -> MHA -> out proj).

Sharding: data-parallel over (batch, query-half): 8 shards for B=4.
Each core gets a [1024, 1024] query-token slice and the full [2048, 768]
context for its batch, and produces a [1024, 1024] output slice.

v2 design notes (engine-stream oriented; the ACT engine runs ONLY the 512
softmax exps [128,512] so its 1.2G elem/s throughput is never wasted):
  - LN on DVE (bn_stats/bn_aggr); gamma/beta broadcast rows; one BATCHED
    xbar-transpose DMA per token tile ([128,C] -> [128, C/128, 128] blocks).
  - Projections: weight-stationary bf16 matmuls, biases applied on DVE
    (tensor_scalar_add), never on ACT.
  - Attention per head pair (even head rows 0-63, odd rows 64-127 of the
    kT chunk -> score matmuls land on disjoint PE row groups and overlap).
    V carries an appended ones column so the attended matmul also emits the
    softmax denominator (psum row 64).  exp on ACT straight out of PSUM
    (scores bounded, no max subtraction).  Scores are emitted one kt ahead
    of the attended matmuls so ACT never starves behind the in-order PE.
  - Warmup: head pair (0,1) q2=0 is interleaved with the context chunks so
    exps begin while K/V projections still run.
  - Out-projection qt tiles are interleaved into the q2=1 attention stream
    (PE has slack there; ACT stays the bottleneck).
"""

import numpy as np

import concourse.bass as bass
import concourse.tile as tile
from concourse import mybir
from concourse.bass_utils import run_bass_kernel_spmd

F32 = mybir.dt.float32
BF16 = mybir.dt.bfloat16
AF = mybir.ActivationFunctionType
OP = mybir.AluOpType

B, NQ_FULL, NK, CQ, CK, H, D = 4, 2048, 2048, 1024, 768, 16, 64
NQ = 1024            # per-core query tokens
N_CORES = 8
EPS = 1e-5
SM_SCALE = 1.0 / np.sqrt(D)  # 0.125

KC_Q = CQ // 128     # 8  contraction chunks for CQ
KC_C = CK // 128     # 6  contraction chunks for CK
NQT = NQ // 128      # 8  query token tiles
NKT = NK // 128      # 16 context token tiles
QC = 512             # psum free-dim limit (fp32)
NQ2 = NQ // QC       # 2


def _split_excess_waits(nc, max_waits=1):
    """walrus in this container accepts at most one sync wait per
    instruction; Tile's kernel-tail drain carries several.  Hoist excess
    waits onto single-wait NOPs that precede the instruction on the same
    engine (absolute sem waits commute, so this is semantics-preserving)."""
    for fn in nc.m.functions:
        for blk in fn.blocks:
            out = []
            dirty = False
            for inst in list(blk.instructions):
                si = inst.sync_info
                if si is not None and len(si.on_wait) > max_waits:
                    waits = list(si.on_wait)
                    for k, w in enumerate(waits[:-max_waits]):
                        nop = mybir.InstNoOp(
                            name=f"wsplit-{inst.name}-{k}", ins=[], outs=[])
                        nop.engine = inst.engine
                        nop.sync_info = mybir.SyncInfo(on_wait=[w], on_update=[])
                        out.append(nop)
                    inst.sync_info = mybir.SyncInfo(
                        on_wait=waits[-max_waits:], on_update=list(si.on_update))
                    dirty = True
                out.append(inst)
            if dirty:
                blk.instructions = out


def _bcast_ap(handle, n_parts, n_free):
    """DRAM [n_free] vector replicated across n_parts partitions."""
    return bass.AP(tensor=handle.ap().tensor, offset=0,
                   ap=[[0, n_parts], [1, n_free]])


def _emit(tc, t, out, mode="full"):
    from contextlib import ExitStack
    nc = tc.nc

    es = ExitStack()
    persist = es.enter_context(tc.tile_pool(name="persist", bufs=1))
    wp = es.enter_context(tc.tile_pool(name="wp", bufs=1))
    wfp = es.enter_context(tc.tile_pool(name="wfp", bufs=2))
    lnp = es.enter_context(tc.tile_pool(name="lnp", bufs=2))
    stp = es.enter_context(tc.tile_pool(name="stp", bufs=4))
    xTp = es.enter_context(tc.tile_pool(name="xTp", bufs=1))
    ep = es.enter_context(tc.tile_pool(name="ep", bufs=2))
    rp = es.enter_context(tc.tile_pool(name="rp", bufs=2))
    scr = es.enter_context(tc.tile_pool(name="scr", bufs=2, space="DRAM"))
    osp = es.enter_context(tc.tile_pool(name="osp", bufs=2))
    pps = es.enter_context(tc.tile_pool(name="pps", bufs=2, space="PSUM"))
    scps = es.enter_context(tc.tile_pool(name="scps", bufs=2, space="PSUM"))
    attps = es.enter_context(tc.tile_pool(name="attps", bufs=2, space="PSUM"))

    # ---- persistent tensors ----
    qTc = [persist.tile([128, NQ], BF16, tag=f"qT{oc}", name=f"qT{oc}")
           for oc in range(KC_Q)]
    kTc = [[persist.tile([128, QC], BF16, tag=f"kT{oc}_{t4}",
                         name=f"kT{oc}_{t4}") for t4 in range(NK // QC)]
           for oc in range(KC_Q)]
    # V with a ones column per head: attended matmul also emits denominator
    v_g = [persist.tile([128, 4, H, D + 1], BF16, tag=f"v{g}", name=f"v{g}")
           for g in range(NKT // 4)]
    attT = persist.tile([128, KC_Q, NQ], BF16, name="attT")
    bq_cols = persist.tile([128, KC_Q], F32)
    bk_cols = persist.tile([128, KC_Q], F32)
    bvb = persist.tile([128, CQ], BF16)
    bob = persist.tile([128, CQ], F32)
    eps_t = persist.tile([128, 1], F32)
    gq_cols = persist.tile([128, KC_Q], F32)
    bqg_cols = persist.tile([128, KC_Q], F32)
    gc_cols = persist.tile([128, KC_C], F32)
    bcg_cols = persist.tile([128, KC_C], F32)

    nc.vector.memset(eps_t[:, :], EPS)
    nc.scalar.dma_start(out=bq_cols[:, :],
                        in_=t["bq"].ap().rearrange("(j p) -> p j", p=128))
    nc.scalar.dma_start(out=bk_cols[:, :],
                        in_=t["bk"].ap().rearrange("(j p) -> p j", p=128))
    # gamma/beta as per-chunk per-partition columns (applied after the
    # transpose, where channel is the partition axis)
    nc.scalar.dma_start(out=gq_cols[:, :],
                        in_=t["gamma_q"].ap().rearrange("(j p) -> p j", p=128))
    nc.scalar.dma_start(out=bqg_cols[:, :],
                        in_=t["beta_q"].ap().rearrange("(j p) -> p j", p=128))
    nc.scalar.dma_start(out=gc_cols[:, :],
                        in_=t["gamma_ctx"].ap().rearrange("(j p) -> p j", p=128))
    nc.scalar.dma_start(out=bcg_cols[:, :],
                        in_=t["beta_ctx"].ap().rearrange("(j p) -> p j", p=128))
    nc.gpsimd.dma_start(out=bvb[:, :], in_=_bcast_ap(t["bv"], 128, CQ))
    nc.gpsimd.dma_start(out=bob[:, :], in_=_bcast_ap(t["bo"], 128, CQ))

    # ---- weights: fp32 staging -> bf16 on ACT (idle during the ramp;
    # Copy is emitted before any Exp).  wq and wo share slots (tag wA) ----
    def load_w(dram, kc, tag, nm, eng, cvt):
        wf = wfp.tile([128, CQ], F32, tag="wf", name=f"wf_{nm}")
        eng.dma_start(out=wf[:, :], in_=dram.ap()[kc * 128:(kc + 1) * 128, :])
        wb = wp.tile([128, CQ], BF16, tag=tag, bufs=8 if tag == "wA" else 6,
                     name=nm)
        cvt(wb, wf)
        return wb

    def cvt_gp(wb, wf):
        nc.gpsimd.tensor_copy(out=wb[:, :], in_=wf[:, :])

    def cvt_dve(wb, wf):
        nc.vector.tensor_copy(out=wb[:, :], in_=wf[:, :])

    wq = [load_w(t["Wq"], kc, "wA", f"wq{kc}",
                 nc.scalar if kc < 4 else nc.sync, cvt_gp)
          for kc in range(KC_Q)]
    wk = [load_w(t["Wk"], kc, "wk", f"wk{kc}",
                 nc.scalar if kc < 3 else nc.sync, cvt_gp)
          for kc in range(KC_C)]
    wv = [load_w(t["Wv"], kc, "wv", f"wv{kc}",
                 nc.scalar if kc < 3 else nc.sync, cvt_gp)
          for kc in range(KC_C)]

    # ---- LN one [128, C] token tile -> bf16 -> one batched transpose;
    # gamma/beta are applied post-transpose as per-partition scalars ----
    def ln_tile(x_dram, i, C, n_sub, sub, xT):
        xf = lnp.tile([128, CQ], F32, tag="xf", name=f"xf_{i}_{C}")
        nc.scalar.dma_start(out=xf[:, 0:C],
                            in_=x_dram.ap()[i * 128:(i + 1) * 128, :])
        st = stp.tile([128, 3, 6], F32, tag="st", name=f"st_{i}_{C}")
        for s in range(n_sub):
            nc.vector.bn_stats(out=st[:, s, :],
                               in_=xf[:, s * sub:(s + 1) * sub])
        mv = stp.tile([128, 2], F32, tag="mv", name=f"mv_{i}_{C}")
        nc.vector.bn_aggr(out=mv[:, :], in_=st[:, 0:n_sub, :])
        nc.scalar.activation(out=mv[:, 1:2], in_=mv[:, 1:2],
                             func=AF.Sqrt, bias=eps_t[:, :], scale=1.0)
        nc.vector.reciprocal(out=mv[:, 1:2], in_=mv[:, 1:2])
        xbf = lnp.tile([128, CQ], BF16, tag="xbf", name=f"xbf_{i}_{C}")
        nc.vector.tensor_scalar(out=xbf[:, 0:C], in0=xf[:, 0:C],
                                scalar1=mv[:, 0:1], scalar2=mv[:, 1:2],
                                op0=OP.subtract, op1=OP.mult)
        # one xbar transpose for all C/128 channel chunks of this tile
        nc.sync.dma_start(out=xT[:, :, (i % 4) * 128:(i % 4) * 128 + 128],
                          in_=xbf[:, 0:C], transpose=True)

    # ---- query side (t2=1 deferred into the ctx loop: warmup only
    # needs tokens 0-511) ----
    def q_chunk(t2):
        xTq = xTp.tile([128, KC_Q, QC], BF16, tag="xTq", name=f"xTq_{t2}")
        for i in range(4):
            ln_tile(t["xq"], t2 * 4 + i, CQ, 2, 512, xTq)
        for kc in range(KC_Q):
            nc.vector.tensor_scalar(out=xTq[:, kc, :], in0=xTq[:, kc, :],
                                    scalar1=gq_cols[:, kc:kc + 1],
                                    scalar2=bqg_cols[:, kc:kc + 1],
                                    op0=OP.mult, op1=OP.add)
        for oc in range(KC_Q):
            ps = pps.tile([128, QC], F32, tag="pp", name=f"psq{oc}_{t2}")
            for kc in range(KC_Q):
                nc.tensor.matmul(ps[:, :],
                                 wq[kc][:, oc * 128:(oc + 1) * 128],
                                 xTq[:, kc, :],
                                 start=(kc == 0), stop=(kc == KC_Q - 1))
            nc.vector.tensor_scalar_add(
                out=qTc[oc][:, t2 * QC:(t2 + 1) * QC], in0=ps[:, :],
                scalar1=bq_cols[:, oc:oc + 1])

    q_chunk(0)

    # ---- attention helpers ----
    def attn_scores_kt(hp, q2, kt, sc):
        g, r = kt // 4, kt % 4
        for par in range(2):
            lo = par * 64
            nc.tensor.matmul(
                sc[:, par * QC:(par + 1) * QC],
                kTc[hp][g][lo:lo + 64, r * 128:(r + 1) * 128],
                qTc[hp][lo:lo + 64, q2 * QC:(q2 + 1) * QC],
                start=True, stop=True)
        # one exp covers both heads' scores (adjacent psum banks)
        e = ep.tile([128, 2 * QC], BF16, tag="e", name=f"e{hp}_{q2}_{kt}")
        nc.scalar.activation(out=e[:, :], in_=sc[:, :],
                             func=AF.Exp, scale=SM_SCALE)
        return e

    def attn_attend_kt(hp, kt, att_pair, e):
        g, r = kt // 4, kt % 4
        for par in range(2):
            h = 2 * hp + par
            nc.tensor.matmul(att_pair[par][:, :],
                             v_g[g][:, r, h, :],
                             e[:, par * QC:(par + 1) * QC],
                             start=(kt == 0), stop=(kt == NKT - 1))

    def attn_normalize(hp, q2, att_pair):
        for par in range(2):
            h = 2 * hp + par
            atc = rp.tile([64, QC], F32, tag="atc", name=f"atc{h}_{q2}")
            nc.vector.tensor_copy(out=atc[:, :], in_=att_pair[par][0:D, :])
            # comb row 64 = reciprocal of denominator; rows 0-63 = its
            # partition-broadcast (DRAM bounce) — one tile for both
            comb = rp.tile([65, QC], F32, tag="comb", name=f"comb{h}_{q2}")
            nc.vector.reciprocal(out=comb[64:65, :],
                                 in_=att_pair[par][64:65, :])
            sd = scr.tile([1, QC], F32, tag="sd", name=f"sd{h}_{q2}")
            nc.gpsimd.dma_start(out=sd[:, :], in_=comb[64:65, :])
            nc.gpsimd.dma_start(
                out=comb[0:64, :],
                in_=bass.AP(tensor=sd.tensor, offset=sd.offset,
                            ap=[[0, 64], [1, QC]]))
            if par == 0:
                nc.vector.tensor_mul(
                    out=attT[0:64, hp, q2 * QC:(q2 + 1) * QC],
                    in0=atc[:, :], in1=comb[0:64, :])
            else:
                tm = rp.tile([64, QC], BF16, tag="tm", bufs=1,
                             name=f"tm{h}_{q2}")
                nc.vector.tensor_mul(out=tm[:, :], in0=atc[:, :],
                                     in1=comb[0:64, :])
                nc.sync.dma_start(
                    out=attT[64:128, hp, q2 * QC:(q2 + 1) * QC], in_=tm[:, :])

    def attn_pair_block(hp, q2, kts, att_pair, pending, last_kt):
        """software-pipelined: scores(kt) emitted before attended(kt-1)."""
        for kt in kts:
            sc = scps.tile([128, 2 * QC], F32, tag="sc",
                           name=f"sc{hp}_{q2}_{kt}")
            e = attn_scores_kt(hp, q2, kt, sc)
            if pending is not None:
                attn_attend_kt(hp, pending[0], att_pair, pending[1])
            pending = (kt, e)
        if kts and kts[-1] == last_kt:
            attn_attend_kt(hp, pending[0], att_pair, pending[1])
            attn_normalize(hp, q2, att_pair)
            pending = None
        return pending

    # warmup attention state: head pair (0,1), q2=0, runs chunk by chunk
    do_attn = mode != "proj"
    att_w = [attps.tile([D + 1, QC], F32, tag="att", name=f"attw{par}")
             for par in range(2)] if do_attn else None
    pend_w = None

    # ---- context side, chunk by chunk, warmup attention interleaved ----
    for t4 in range(NK // QC):
        xTc = xTp.tile([128, KC_C, QC], BF16, tag="xTc", bufs=2,
                       name=f"xTc_{t4}")
        for i in range(4):
            ln_tile(t["xc"], t4 * 4 + i, CK, 3, 256, xTc)
        for kc in range(KC_C):
            nc.vector.tensor_scalar(out=xTc[:, kc, :], in0=xTc[:, kc, :],
                                    scalar1=gc_cols[:, kc:kc + 1],
                                    scalar2=bcg_cols[:, kc:kc + 1],
                                    op0=OP.mult, op1=OP.add)
        for oc in range(KC_Q):
            ps = pps.tile([128, QC], F32, tag="pp", name=f"psk{oc}_{t4}")
            for kc in range(KC_C):
                nc.tensor.matmul(ps[:, :],
                                 wk[kc][:, oc * 128:(oc + 1) * 128],
                                 xTc[:, kc, :],
                                 start=(kc == 0), stop=(kc == KC_C - 1))
            nc.vector.tensor_scalar_add(out=kTc[oc][t4][:, :], in0=ps[:, :],
                                        scalar1=bk_cols[:, oc:oc + 1])
        for ki in range(4):
            for v2 in range(CQ // QC):
                ps = pps.tile([128, QC], F32, tag="pp", name=f"psv{t4}_{ki}{v2}")
                for kc in range(KC_C):
                    nc.tensor.matmul(ps[:, :],
                                     xTc[:, kc, ki * 128:(ki + 1) * 128],
                                     wv[kc][:, v2 * QC:(v2 + 1) * QC],
                                     start=(kc == 0), stop=(kc == KC_C - 1))
                nc.vector.tensor_tensor(
                    out=v_g[t4][:, ki, v2 * 8:(v2 + 1) * 8, 0:D],
                    in0=ps[:, :].rearrange("p (h d) -> p h d", d=D),
                    in1=bvb[:, v2 * QC:(v2 + 1) * QC].rearrange(
                        "p (h d) -> p h d", d=D),
                    op=OP.add)
            nc.vector.memset(v_g[t4][:, ki, :, D:D + 1], 1.0)
        # warmup: head pair 0 (q2=0) over this chunk's kt range
        if do_attn:
            pend_w = attn_pair_block(0, 0, list(range(4 * t4, 4 * t4 + 4)),
                                     att_w, pend_w, NKT - 1)
        if t4 == 0:
            q_chunk(1)

    # wo loads reuse the wq slots (tag wA); sync queue is quiet by now and
    # the ACT queue must stay exp-only, so stage via sync + convert on DVE
    wo = [load_w(t["Wo"], kc, "wA", f"wo{kc}", nc.sync, cvt_dve)
          for kc in range(KC_Q)]

    # ---- out projection helper (interleaved later) ----
    def outproj_qt(qt):
        for cc in range(CQ // QC):
            ps = pps.tile([128, QC], F32, tag="pp", name=f"pso{qt}_{cc}")
            for kc in range(KC_Q):
                nc.tensor.matmul(
                    ps[:, :],
                    attT[:, kc, qt * 128:(qt + 1) * 128],
                    wo[kc][:, cc * QC:(cc + 1) * QC],
                    start=(kc == 0), stop=(kc == KC_Q - 1))
            osb = osp.tile([128, QC], F32, tag="osb", bufs=1,
                           name=f"osb{qt}_{cc}")
            nc.vector.tensor_tensor(out=osb[:, :], in0=ps[:, :],
                                    in1=bob[:, cc * QC:(cc + 1) * QC],
                                    op=OP.add)
            nc.sync.dma_start(
                out=out.ap()[qt * 128:(qt + 1) * 128, cc * QC:(cc + 1) * QC],
                in_=osb[:, :])

    if mode == "proj":
        fb = osp.tile([128, QC], F32, tag="osb", bufs=1, name="fb")
        nc.vector.tensor_copy(out=fb[:, :], in_=qTc[0][:, 0:QC])
        nc.sync.dma_start(out=out.ap()[0:128, 0:QC], in_=fb[:, :])
        es.close()
        return

    # ---- steady attention: q2=0 pairs 1..7, then q2=1 pairs 0..7 with
    # out-projection qt tiles interleaved into the q2=1 stream ----
    for hp in range(1, H // 2):
        att_pair = [attps.tile([D + 1, QC], F32, tag="att",
                               name=f"att{hp}_0_{par}") for par in range(2)]
        attn_pair_block(hp, 0, list(range(NKT)), att_pair, None, NKT - 1)

    for hp in range(H // 2):
        att_pair = [attps.tile([D + 1, QC], F32, tag="att",
                               name=f"att{hp}_1_{par}") for par in range(2)]
        attn_pair_block(hp, 1, list(range(NKT)), att_pair, None, NKT - 1)
        if hp >= 4:
            outproj_qt(hp - 4)          # qt 0..3 (q2=0 halves) overlap
    for qt in range(4, NQT):
        outproj_qt(qt)

    es.close()


def build():
    nc = bass.Bass("TRN2", target_bir_lowering=False, debug=False,
                   num_devices=N_CORES)
    t = {
        "xq": nc.dram_tensor("xq", [NQ, CQ], F32, kind="ExternalInput"),
        "xc": nc.dram_tensor("xc", [NK, CK], F32, kind="ExternalInput"),
        "Wq": nc.dram_tensor("Wq", [CQ, CQ], F32, kind="ExternalInput"),
        "Wk": nc.dram_tensor("Wk", [CK, CQ], F32, kind="ExternalInput"),
        "Wv": nc.dram_tensor("Wv", [CK, CQ], F32, kind="ExternalInput"),
        "Wo": nc.dram_tensor("Wo", [CQ, CQ], F32, kind="ExternalInput"),
        "bq": nc.dram_tensor("bq", [CQ], F32, kind="ExternalInput"),
        "bk": nc.dram_tensor("bk", [CQ], F32, kind="ExternalInput"),
        "bv": nc.dram_tensor("bv", [CQ], F32, kind="ExternalInput"),
        "bo": nc.dram_tensor("bo", [CQ], F32, kind="ExternalInput"),
        "gamma_q": nc.dram_tensor("gamma_q", [CQ], F32, kind="ExternalInput"),
        "beta_q": nc.dram_tensor("beta_q", [CQ], F32, kind="ExternalInput"),
        "gamma_ctx": nc.dram_tensor("gamma_ctx", [CK], F32, kind="ExternalInput"),
        "beta_ctx": nc.dram_tensor("beta_ctx", [CK], F32, kind="ExternalInput"),
    }
    out = nc.dram_tensor("out", [NQ, CQ], F32, kind="ExternalOutput")
    with tile.TileContext(nc) as tc:
        _emit(tc, t, out)
    _split_excess_waits(nc)
    return nc


_NC = None


def _in_maps(inputs):
    q = np.ascontiguousarray(np.asarray(inputs["query_tokens"], dtype=np.float32))
    c = np.ascontiguousarray(np.asarray(inputs["context_tokens"], dtype=np.float32))
    shared = {k: np.ascontiguousarray(np.asarray(inputs[k], dtype=np.float32))
              for k in ("Wq", "Wk", "Wv", "Wo", "bq", "bk", "bv", "bo",
                        "gamma_q", "beta_q", "gamma_ctx", "beta_ctx")}
    maps = []
    for core in range(N_CORES):
        b, half = core // 2, core % 2
        m = dict(shared)
        m["xq"] = np.ascontiguousarray(q[b, half * NQ:(half + 1) * NQ, :])
        m["xc"] = np.ascontiguousarray(c[b])
        maps.append(m)
    return maps


def run_sharded(inputs, **kwargs):
    global _NC
    if _NC is None:
        _NC = build()
    return run_bass_kernel_spmd(_NC, _in_maps(inputs),
                                core_ids=list(range(N_CORES)), **kwargs)


def kernel(**inputs) -> np.ndarray:
    res = run_sharded(inputs)
    out = np.empty((B, NQ_FULL, CQ), np.float32)
    for core in range(N_CORES):
        b, half = core // 2, core % 2
        out[b, half * NQ:(half + 1) * NQ, :] = res.results[core]["out"]
    return out
